# revision 11
# baseline (speedup 1.0000x reference)
"""Causal MHSA (pre-LN, relative position bias, residual) on 8 Trainium2 cores.

Sharding: batch (4) x head-half (2) -> 8 cores. Core c handles batch c//2 and
heads (c%2)*8 .. (c%2)*8+8. Host sums the two per-batch partials and adds the
residual.

Layout (per core, T=2048, D=1024, dh=64, NH=8, M=512):
  - No small DMA transposes (1.3us fixed cost each killed the original):
    x^T is produced by 16 *large* dma_start_transpose ops (one per 128-row
    tile, 3D strided output) into a single [128, 8, 2048] xcsT tile.
  - Scores are computed directly in transposed [j, i] layout
    (S^T = K^T Q via lhsT=k-slice, rhs=q-slice) so P^T feeds the AV matmul
    with no transpose of P.  Two j-tiles share one [128,1024] PSUM tile and
    one Exp activation (halves ACT instruction count).
  - The rel-pos band + causal mask are applied POST-exp as a multiply by
    host-precomputed exp(band) (zeros above the diagonal), in fast
    fp16-SBUF DVE mode, so the (ACT-bound) exp does not wait on any
    vector-engine work.
  - The softmax denominator Z[i] is accumulated by a ones-column appended
    to V: row 64 of U' = V'^T P^T.  1/Z is partition-broadcast on gpsimd and
    multiplied in on DVE.
  - q/k projections for head-rows 2..7 are emitted *interleaved* with the
    attention of earlier heads as PE filler, so the tensor engine stays busy
    while the scalar engine grinds through exps.
  out[t, d] = yT as lhsT @ woT   (partial; host adds pair + residual)
"""

import math
import sys

sys.path.insert(0, "/opt/trn_rl_repo")

import numpy as np
from contextlib import ExitStack

import concourse.bacc as bacc
import concourse.tile as tile
import concourse.mybir as mybir
from concourse.bass_utils import run_bass_kernel_spmd

F32 = mybir.dt.float32
F16 = mybir.dt.float16

T = 2048
D = 1024
DH = 64
NH = 8  # heads per core
M = NH * DH  # 512 head-dims per core
TT = T // 128  # 16 token tiles
DT = D // 128  # 8 d-chunks
MT = M // 128  # 4 m-tiles
NCORES = 8
LN_EPS = 1e-5

_CACHED_NC = None


def build_nc():
    nc = bacc.Bacc("TRN2", target_bir_lowering=False, debug=False, num_devices=NCORES)

    x_d = nc.dram_tensor("x", [T, D], F32, kind="ExternalInput")
    wqT_d = nc.dram_tensor("wqT", [D, M], F16, kind="ExternalInput")
    wkT_d = nc.dram_tensor("wkT", [D, M], F16, kind="ExternalInput")
    wvT_d = nc.dram_tensor("wvT", [D, M], F16, kind="ExternalInput")
    woT_d = nc.dram_tensor("woT", [M, D], F16, kind="ExternalInput")
    ebandT_d = nc.dram_tensor("ebandT", [128, NH * 256], F16, kind="ExternalInput")
    bq_d = nc.dram_tensor("bq", [128, MT], F32, kind="ExternalInput")
    bk_d = nc.dram_tensor("bk", [128, MT], F32, kind="ExternalInput")
    bv_d = nc.dram_tensor("bv", [128, NH, DH], F16, kind="ExternalInput")
    out_d = nc.dram_tensor("out", [T, D], F32, kind="ExternalOutput")

    with tile.TileContext(nc) as tc, ExitStack() as ctx:
        singles = ctx.enter_context(tc.tile_pool(name="singles", bufs=1))
        xload = ctx.enter_context(tc.tile_pool(name="xload", bufs=3))
        stats = ctx.enter_context(tc.tile_pool(name="stats", bufs=6))
        xcs = ctx.enter_context(tc.tile_pool(name="xcs", bufs=3))
        xcsTp = ctx.enter_context(tc.tile_pool(name="xcsTp", bufs=1))
        wt = ctx.enter_context(tc.tile_pool(name="wt", bufs=1))
        qkT = ctx.enter_context(tc.tile_pool(name="qkT", bufs=1))
        vpool = ctx.enter_context(tc.tile_pool(name="vpool", bufs=1))
        pts = ctx.enter_context(tc.tile_pool(name="pts", bufs=3))
        rzp = ctx.enter_context(tc.tile_pool(name="rzp", bufs=2))
        rzbp = ctx.enter_context(tc.tile_pool(name="rzbp", bufs=2))
        ypool = ctx.enter_context(tc.tile_pool(name="ypool", bufs=1))
        outp = ctx.enter_context(tc.tile_pool(name="outp", bufs=4))

        psS = ctx.enter_context(tc.tile_pool(name="psS", bufs=2, space="PSUM"))
        psU = ctx.enter_context(tc.tile_pool(name="psU", bufs=2, space="PSUM"))
        psP = ctx.enter_context(tc.tile_pool(name="psP", bufs=2, space="PSUM"))

        # ---- singles ----
        ebandT_sb = singles.tile([128, NH * 256], F16)
        nc.sync.dma_start(out=ebandT_sb, in_=ebandT_d[:, :])
        bq_sb = singles.tile([128, MT], F32)
        nc.sync.dma_start(out=bq_sb, in_=bq_d[:, :])
        bk_sb = singles.tile([128, MT], F32)
        nc.sync.dma_start(out=bk_sb, in_=bk_d[:, :])
        bv_sb = singles.tile([128, NH, DH], F16)
        nc.sync.dma_start(out=bv_sb, in_=bv_d[:, :, :])
        eps_sb = singles.tile([128, 1], F32)
        nc.vector.memset(eps_sb, LN_EPS)

        # weights up front
        wqs, wks, wvs = [], [], []
        for pi, (w_d, lst) in enumerate(((wqT_d, wqs), (wkT_d, wks), (wvT_d, wvs))):
            for d in range(DT):
                wtd = wt.tile([128, M], F16, name=f"w{pi}_{d}", tag=f"w{pi}_{d}")
                nc.sync.dma_start(out=wtd, in_=w_d[d * 128 : (d + 1) * 128, :])
                lst.append(wtd)
        wos = []
        for kt in range(MT):
            wod = wt.tile([128, D], F16, name=f"wo{kt}", tag=f"wo_{kt}")
            nc.sync.dma_start(out=wod, in_=woT_d[kt * 128 : (kt + 1) * 128, :])
            wos.append(wod)

        # ---- phase 1: layernorm (center+scale), big DMA transpose -> xcsT ----
        # xcsT[p, c, t] = xcs[t, c*128 + p]
        xcsT_b = xcsTp.tile([128, DT, T], F16, name="xcsT_b")
        for tt in range(TT):
            xt = xload.tile([128, D], F32, tag="xt")
            nc.sync.dma_start(out=xt, in_=x_d[tt * 128 : (tt + 1) * 128, :])
            st6 = stats.tile([128, 2, 6], F32, tag="st6")
            nc.vector.bn_stats(out=st6[:, 0, :], in_=xt[:, 0:512])
            nc.vector.bn_stats(out=st6[:, 1, :], in_=xt[:, 512:1024])
            mv = stats.tile([128, 2], F32, tag="mv")
            nc.vector.bn_aggr(out=mv, in_=st6)
            sq = stats.tile([128, 1], F32, tag="sq")
            nc.scalar.activation(
                out=sq, in_=mv[:, 1:2], func=mybir.ActivationFunctionType.Sqrt,
                bias=eps_sb[:, :], scale=1.0,
            )
            rstd = stats.tile([128, 1], F32, tag="rstd")
            nc.vector.reciprocal(out=rstd, in_=sq)
            xcs_t = xcs.tile([128, D], F16, tag="xcs")
            nc.gpsimd.tensor_scalar(
                out=xcs_t, in0=xt, scalar1=mv[:, 0:1], scalar2=rstd,
                op0=mybir.AluOpType.subtract, op1=mybir.AluOpType.mult,
            )
            nc.sync.dma_start_transpose(
                out=xcsT_b[:, :, tt * 128 : (tt + 1) * 128], in_=xcs_t
            )

        def xT(d, lo, hi):
            return xcsT_b[:, d, lo:hi]

        # ---- phase 2 emission helpers ----
        qkT_t = [qkT.tile([128, T], F16, name=f"qkT{i}") for i in range(2 * MT)]

        def qk_proj_steps(mt):
            """Generator of per-matmul steps for the q and k projections of
            head-row mt (heads 2mt, 2mt+1)."""
            for pi, (wts, b_sb) in enumerate(((wqs, bq_sb), (wks, bk_sb))):
                for tc4 in range(4):
                    ps = psP.tile([128, 512], F32, tag="p")
                    for d in range(DT):
                        yield lambda ps=ps, d=d, wts=wts, mt=mt, tc4=tc4: nc.tensor.matmul(
                            ps,
                            lhsT=wts[d][:, mt * 128 : (mt + 1) * 128],
                            rhs=xT(d, tc4 * 512, (tc4 + 1) * 512),
                            start=(d == 0), stop=(d == DT - 1),
                        )
                    yield lambda ps=ps, pi=pi, b_sb=b_sb, mt=mt, tc4=tc4: nc.vector.tensor_scalar(
                        out=qkT_t[pi * MT + mt][:, tc4 * 512 : (tc4 + 1) * 512],
                        in0=ps, scalar1=b_sb[:, mt : mt + 1], scalar2=None,
                        op0=mybir.AluOpType.add,
                    )

        # q/k projections for head-row 0 up front; rows 1..3 become PE filler
        # inside the attention loop below.
        for step in qk_proj_steps(0):
            step()

        # ---- phase 2b: v projection -> v' [t, h, 65] f16 (ones col at 64) ----
        v_t = [vpool.tile([128, NH, DH + 1], F16, name=f"v{tt}") for tt in range(TT)]
        for tt in range(TT):
            ps = psP.tile([128, 512], F32, tag="p")
            for d in range(DT):
                nc.tensor.matmul(
                    ps,
                    lhsT=xT(d, tt * 128, (tt + 1) * 128),
                    rhs=wvs[d],
                    start=(d == 0), stop=(d == DT - 1),
                )
            nc.gpsimd.memset(v_t[tt][:, :, DH : DH + 1], 1.0)
            nc.vector.tensor_add(
                out=v_t[tt][:, :, 0:DH],
                in0=ps.rearrange("p (h d) -> p h d", d=DH),
                in1=bv_sb,
            )
        filler = []
        for mt in range(1, MT):
            filler.extend(qk_proj_steps(mt))
        fstate = {"i": 0}

        def pull_filler(n):
            while fstate["i"] < len(filler) and n > 0:
                filler[fstate["i"]]()
                fstate["i"] += 1
                n -= 1

        def flush_filler_through(mt):
            """Emit all remaining filler steps for head-rows <= mt."""
            target = mt * len(filler) // (MT - 1)
            while fstate["i"] < target:
                filler[fstate["i"]]()
                fstate["i"] += 1

        # ---- phase 3: attention per (head, 512-wide i-block) ----
        yT_t = [ypool.tile([128, T], F16, name=f"yT{i}") for i in range(MT)]
        for h in range(NH):
            qrow = h // 2
            roff = (h % 2) * 64
            if qrow >= 1:
                flush_filler_through(qrow)
            kT_ap = qkT_t[MT + qrow]
            qT_ap = qkT_t[qrow]
            for ib in range(4):
                jlast = 4 * ib + 3
                Ups = psU.tile([128, 512], F32, tag="u")
                pend = []  # (jt, pt_tile, half_off, c0)
                for j0 in range(0, jlast + 1, 2):
                    ps = psS.tile([128, 1024], F32, tag="s")
                    pt = pts.tile([128, 1024], F16, tag="pt")
                    c0s = []
                    for half, jj in ((0, j0), (512, j0 + 1)):
                        k = jj - 4 * ib
                        c0 = 128 * k if k > 0 else 0
                        c0s.append(c0)
                        nc.tensor.matmul(
                            ps[:, half + c0 : half + 512],
                            lhsT=kT_ap[roff : roff + 64, jj * 128 : (jj + 1) * 128],
                            rhs=qT_ap[roff : roff + 64, ib * 512 + c0 : (ib + 1) * 512],
                            start=True, stop=True,
                        )
                    nc.scalar.activation(
                        out=pt[:, c0s[0] : 1024],
                        in_=ps[:, c0s[0] : 1024],
                        func=mybir.ActivationFunctionType.Exp,
                        bias=0.0, scale=1.0,
                    )
                    # post-exp band multiply (rel bias + causal zeros), f16 SBUF
                    for half, jj in ((0, j0), (512, j0 + 1)):
                        k = jj - 4 * ib
                        if -1 <= k <= 3:
                            lo = max(0, 128 * k)
                            hi = min(512, 128 * k + 256)
                            eb0 = h * 256 + (0 if k >= 0 else 128)
                            nc.vector.tensor_mul(
                                out=pt[:, half + lo : half + hi],
                                in0=pt[:, half + lo : half + hi],
                                in1=ebandT_sb[:, eb0 : eb0 + (hi - lo)],
                            )
                    pend.append((j0, pt, 0, c0s[0]))
                    pend.append((j0 + 1, pt, 512, c0s[1]))
                    pull_filler(2)
                    while len(pend) > 2:
                        jj, apt, half, cc = pend.pop(0)
                        nc.tensor.matmul(
                            Ups[0 : DH + 1, cc:512],
                            lhsT=v_t[jj][:, h, :],
                            rhs=apt[:, half + cc : half + 512],
                            start=(jj == 0), stop=False,
                            skip_group_check=True,
                        )
                for jj, apt, half, cc in pend:
                    nc.tensor.matmul(
                        Ups[0 : DH + 1, cc:512],
                        lhsT=v_t[jj][:, h, :],
                        rhs=apt[:, half + cc : half + 512],
                        start=(jj == 0), stop=(jj == jlast),
                        skip_group_check=True,
                    )
                rz = rzp.tile([1, 512], F32, tag="rz")
                nc.vector.reciprocal(out=rz, in_=Ups[DH : DH + 1, :])
                rzb = rzbp.tile([64, 512], F32, tag="rzb")
                nc.gpsimd.partition_broadcast(rzb, rz)
                nc.vector.tensor_mul(
                    out=yT_t[qrow][roff : roff + 64, ib * 512 : (ib + 1) * 512],
                    in0=Ups[0:DH, :],
                    in1=rzb,
                )
        flush_filler_through(MT - 1)

        # ---- phase 4: output projection (partial; host adds residual) ----
        for tt in range(TT):
            for oc in range(2):
                ps = psP.tile([128, 512], F32, tag="p")
                for kt in range(MT):
                    nc.tensor.matmul(
                        ps,
                        lhsT=yT_t[kt][:, tt * 128 : (tt + 1) * 128],
                        rhs=wos[kt][:, oc * 512 : (oc + 1) * 512],
                        start=(kt == 0), stop=(kt == MT - 1),
                    )
                osb = outp.tile([128, 512], F32, tag="o")
                if oc == 0:
                    nc.scalar.copy(out=osb, in_=ps)
                else:
                    nc.vector.tensor_copy(out=osb, in_=ps)
                nc.sync.dma_start(
                    out=out_d[tt * 128 : (tt + 1) * 128, oc * 512 : (oc + 1) * 512],
                    in_=osb,
                )

    nc.compile()
    return nc


def _host_prep(inputs):
    """Build the 8 per-core input maps."""
    x = np.asarray(inputs["x"], dtype=np.float32)
    Wq = np.asarray(inputs["Wq"], dtype=np.float32)
    Wk = np.asarray(inputs["Wk"], dtype=np.float32)
    Wv = np.asarray(inputs["Wv"], dtype=np.float32)
    Wo = np.asarray(inputs["Wo"], dtype=np.float32)
    rel = np.asarray(inputs["rel"], dtype=np.float32)
    gamma = np.asarray(inputs["ln_gamma"], dtype=np.float32)
    beta = np.asarray(inputs["ln_beta"], dtype=np.float32)

    sc = 1.0 / math.sqrt(DH)
    ii = np.arange(128)

    half = {}
    for hh in range(2):
        hs = slice(hh * M, (hh + 1) * M)
        Wq_h, Wk_h, Wv_h, Wo_h = Wq[hs], Wk[hs], Wv[hs], Wo[:, hs]
        wqT = np.ascontiguousarray((Wq_h * gamma[None, :] * sc).T).astype(np.float16)
        wkT = np.ascontiguousarray((Wk_h * gamma[None, :]).T).astype(np.float16)
        wvT = np.ascontiguousarray((Wv_h * gamma[None, :]).T).astype(np.float16)
        woT = np.ascontiguousarray(Wo_h.T).astype(np.float16)
        bq = ((Wq_h @ beta) * sc).reshape(MT, 128).T.astype(np.float32)
        bk = (Wk_h @ beta).reshape(MT, 128).T.astype(np.float32)
        bv = np.tile((Wv_h @ beta)[None, :], (128, 1)).reshape(128, NH, DH)

        # transposed multiplicative band tiles [j, i]:
        #   diag (it==jt):    exp(rel[i-j] - rel[128]) for i>=j, 0 above diag
        #   offdiag (it==jt+1): exp(rel[min(i-j+128,128)] - rel[128])
        ebandT = np.zeros((128, NH * 256), dtype=np.float32)
        d0 = ii[None, :] - ii[:, None]  # [j, i] = i - j
        for h in range(NH):
            g = hh * NH + h
            r128 = rel[g, 128]
            diag = np.where(d0 >= 0, np.exp(rel[g, np.clip(d0, 0, 128)] - r128), 0.0)
            offd = np.exp(rel[g, np.minimum(d0 + 128, 128)] - r128)
            ebandT[:, h * 256 : h * 256 + 128] = diag
            ebandT[:, h * 256 + 128 : h * 256 + 256] = offd
        half[hh] = dict(
            wqT=wqT, wkT=wkT, wvT=wvT, woT=woT,
            bq=np.ascontiguousarray(bq), bk=np.ascontiguousarray(bk),
            bv=bv.astype(np.float16),
            ebandT=ebandT.astype(np.float16),
        )

    in_maps = []
    for c in range(NCORES):
        b, hh = c // 2, c % 2
        m = dict(half[hh])
        m["x"] = np.ascontiguousarray(x[b])
        in_maps.append(m)
    return in_maps, x


def kernel(**inputs) -> np.ndarray:
    global _CACHED_NC
    if _CACHED_NC is None:
        _CACHED_NC = build_nc()
    nc = _CACHED_NC
    in_maps, x = _host_prep(inputs)
    res = run_bass_kernel_spmd(nc, in_maps, core_ids=list(range(NCORES)))
    out = np.empty_like(x)
    for b in range(4):
        out[b] = x[b] + res.results[2 * b]["out"] + res.results[2 * b + 1]["out"]
    return out


if __name__ == "__main__":
    rng = np.random.default_rng(0)
    fake = {
        "x": rng.standard_normal((4, T, D), dtype=np.float32),
        "Wq": rng.standard_normal((D, D), dtype=np.float32) / 32,
        "Wk": rng.standard_normal((D, D), dtype=np.float32) / 32,
        "Wv": rng.standard_normal((D, D), dtype=np.float32) / 32,
        "Wo": rng.standard_normal((D, D), dtype=np.float32) / 32,
        "rel": np.tile(np.linspace(0, -2, 129, dtype=np.float32), (16, 1)),
        "ln_gamma": np.ones(D, np.float32),
        "ln_beta": np.zeros(D, np.float32),
    }
    y = kernel(**fake)
    print("ran ok", y.shape, y.dtype)


# revision 12
# speedup vs baseline: 1.3047x; 1.3047x over previous
"""Causal MHSA (pre-LN, relative position bias, residual) on 8 Trainium2 cores.

Sharding: batch (4) x head-half (2) -> 8 cores. Core c handles batch c//2 and
heads (c%2)*8 .. (c%2)*8+8. Host sums the two per-batch partials and adds the
residual.

Layout (per core, T=2048, D=1024, dh=64, NH=8, M=512):
  - No small DMA transposes (1.3us fixed cost each killed the original):
    x^T is produced by 16 *large* dma_start_transpose ops (one per 128-row
    tile, 3D strided output) into a single [128, 8, 2048] xcsT tile.
  - Scores are computed directly in transposed [j, i] layout
    (S^T = K^T Q via lhsT=k-slice, rhs=q-slice) so P^T feeds the AV matmul
    with no transpose of P.  Two j-tiles share one [128,1024] PSUM tile and
    one Exp activation (halves ACT instruction count).
  - The rel-pos band + causal mask are applied POST-exp as a multiply by
    host-precomputed exp(band) (zeros above the diagonal), in fast
    fp16-SBUF DVE mode, so the (ACT-bound) exp does not wait on any
    vector-engine work.
  - The softmax denominator Z[i] is accumulated by a ones-column appended
    to V: row 64 of U' = V'^T P^T.  1/Z is partition-broadcast on gpsimd and
    multiplied in on DVE.
  - q/k projections for head-rows 2..7 are emitted *interleaved* with the
    attention of earlier heads as PE filler, so the tensor engine stays busy
    while the scalar engine grinds through exps.
  out[t, d] = yT as lhsT @ woT   (partial; host adds pair + residual)
"""

import math
import sys

sys.path.insert(0, "/opt/trn_rl_repo")

import numpy as np
from contextlib import ExitStack

import concourse.bacc as bacc
import concourse.tile as tile
import concourse.mybir as mybir
from concourse.bass_utils import run_bass_kernel_spmd

F32 = mybir.dt.float32
F16 = mybir.dt.float16

T = 2048
D = 1024
DH = 64
NH = 8  # heads per core
M = NH * DH  # 512 head-dims per core
TT = T // 128  # 16 token tiles
DT = D // 128  # 8 d-chunks
MT = M // 128  # 4 m-tiles
NCORES = 8
LN_EPS = 1e-5

_CACHED_NC = None


def build_nc():
    nc = bacc.Bacc("TRN2", target_bir_lowering=False, debug=False, num_devices=NCORES)

    x_d = nc.dram_tensor("x", [T, D], F32, kind="ExternalInput")
    wqT_d = nc.dram_tensor("wqT", [D, M], F16, kind="ExternalInput")
    wkT_d = nc.dram_tensor("wkT", [D, M], F16, kind="ExternalInput")
    wvT_d = nc.dram_tensor("wvT", [D, M], F16, kind="ExternalInput")
    woT_d = nc.dram_tensor("woT", [M, D], F16, kind="ExternalInput")
    ebandT_d = nc.dram_tensor("ebandT", [128, NH * 256], F16, kind="ExternalInput")
    bq_d = nc.dram_tensor("bq", [128, MT], F32, kind="ExternalInput")
    bk_d = nc.dram_tensor("bk", [128, MT], F32, kind="ExternalInput")
    bv_d = nc.dram_tensor("bv", [128, NH, DH], F16, kind="ExternalInput")
    out_d = nc.dram_tensor("out", [T, D], F32, kind="ExternalOutput")

    with tile.TileContext(nc) as tc, ExitStack() as ctx:
        singles = ctx.enter_context(tc.tile_pool(name="singles", bufs=1))
        xload = ctx.enter_context(tc.tile_pool(name="xload", bufs=3))
        stats = ctx.enter_context(tc.tile_pool(name="stats", bufs=6))
        xcs = ctx.enter_context(tc.tile_pool(name="xcs", bufs=3))
        xcsTp = ctx.enter_context(tc.tile_pool(name="xcsTp", bufs=1))
        wt = ctx.enter_context(tc.tile_pool(name="wt", bufs=1))
        qkT = ctx.enter_context(tc.tile_pool(name="qkT", bufs=1))
        vpool = ctx.enter_context(tc.tile_pool(name="vpool", bufs=1))
        pts = ctx.enter_context(tc.tile_pool(name="pts", bufs=3))
        rzp = ctx.enter_context(tc.tile_pool(name="rzp", bufs=2))
        rzbp = ctx.enter_context(tc.tile_pool(name="rzbp", bufs=2))
        ypool = ctx.enter_context(tc.tile_pool(name="ypool", bufs=1))
        outp = ctx.enter_context(tc.tile_pool(name="outp", bufs=4))

        psS = ctx.enter_context(tc.tile_pool(name="psS", bufs=2, space="PSUM"))
        psU = ctx.enter_context(tc.tile_pool(name="psU", bufs=2, space="PSUM"))
        psP = ctx.enter_context(tc.tile_pool(name="psP", bufs=2, space="PSUM"))

        # ---- singles ----
        ebandT_sb = singles.tile([128, NH * 256], F16)
        nc.sync.dma_start(out=ebandT_sb, in_=ebandT_d[:, :])
        bq_sb = singles.tile([128, MT], F32)
        nc.sync.dma_start(out=bq_sb, in_=bq_d[:, :])
        bk_sb = singles.tile([128, MT], F32)
        nc.sync.dma_start(out=bk_sb, in_=bk_d[:, :])
        bv_sb = singles.tile([128, NH, DH], F16)
        nc.sync.dma_start(out=bv_sb, in_=bv_d[:, :, :])
        eps_sb = singles.tile([128, 1], F32)
        nc.vector.memset(eps_sb, LN_EPS)

        # weights up front
        wqs, wks, wvs = [], [], []
        for pi, (w_d, lst) in enumerate(((wqT_d, wqs), (wkT_d, wks), (wvT_d, wvs))):
            for d in range(DT):
                wtd = wt.tile([128, M], F16, name=f"w{pi}_{d}", tag=f"w{pi}_{d}")
                nc.sync.dma_start(out=wtd, in_=w_d[d * 128 : (d + 1) * 128, :])
                lst.append(wtd)
        wos = []
        for kt in range(MT):
            wod = wt.tile([128, D], F16, name=f"wo{kt}", tag=f"wo_{kt}")
            nc.sync.dma_start(out=wod, in_=woT_d[kt * 128 : (kt + 1) * 128, :])
            wos.append(wod)

        # ---- phase 1: layernorm (center+scale), big DMA transpose -> xcsT ----
        # xcsT[p, c, t] = xcs[t, c*128 + p]
        xcsT_b = xcsTp.tile([128, DT, T], F16, name="xcsT_b")
        for tt in range(TT):
            xt = xload.tile([128, D], F32, tag="xt")
            nc.sync.dma_start(out=xt, in_=x_d[tt * 128 : (tt + 1) * 128, :])
            st6 = stats.tile([128, 2, 6], F32, tag="st6")
            nc.vector.bn_stats(out=st6[:, 0, :], in_=xt[:, 0:512])
            nc.vector.bn_stats(out=st6[:, 1, :], in_=xt[:, 512:1024])
            mv = stats.tile([128, 2], F32, tag="mv")
            nc.vector.bn_aggr(out=mv, in_=st6)
            sq = stats.tile([128, 1], F32, tag="sq")
            nc.scalar.activation(
                out=sq, in_=mv[:, 1:2], func=mybir.ActivationFunctionType.Sqrt,
                bias=eps_sb[:, :], scale=1.0,
            )
            rstd = stats.tile([128, 1], F32, tag="rstd")
            nc.vector.reciprocal(out=rstd, in_=sq)
            xcs_t = xcs.tile([128, D], F16, tag="xcs")
            nc.vector.tensor_scalar(
                out=xcs_t, in0=xt, scalar1=mv[:, 0:1], scalar2=rstd,
                op0=mybir.AluOpType.subtract, op1=mybir.AluOpType.mult,
            )
            nc.sync.dma_start_transpose(
                out=xcsT_b[:, :, tt * 128 : (tt + 1) * 128], in_=xcs_t
            )

        def xT(d, lo, hi):
            return xcsT_b[:, d, lo:hi]

        # ---- phase 2 emission helpers ----
        qkT_t = [qkT.tile([128, T], F16, name=f"qkT{i}") for i in range(2 * MT)]

        def qk_proj_steps(mt):
            """Generator of per-matmul steps for the q and k projections of
            head-row mt (heads 2mt, 2mt+1)."""
            for pi, (wts, b_sb) in enumerate(((wqs, bq_sb), (wks, bk_sb))):
                for tc4 in range(4):
                    ps = psP.tile([128, 512], F32, tag="p")
                    for d in range(DT):
                        yield lambda ps=ps, d=d, wts=wts, mt=mt, tc4=tc4: nc.tensor.matmul(
                            ps,
                            lhsT=wts[d][:, mt * 128 : (mt + 1) * 128],
                            rhs=xT(d, tc4 * 512, (tc4 + 1) * 512),
                            start=(d == 0), stop=(d == DT - 1),
                        )
                    yield lambda ps=ps, pi=pi, b_sb=b_sb, mt=mt, tc4=tc4: nc.vector.tensor_scalar(
                        out=qkT_t[pi * MT + mt][:, tc4 * 512 : (tc4 + 1) * 512],
                        in0=ps, scalar1=b_sb[:, mt : mt + 1], scalar2=None,
                        op0=mybir.AluOpType.add,
                    )

        # q/k projections for head-row 0 up front; rows 1..3 become PE filler
        # inside the attention loop below.
        for step in qk_proj_steps(0):
            step()

        # ---- phase 2b: v projection -> v' [t, h, 65] f16 (ones col at 64) ----
        v_t = [vpool.tile([128, NH, DH + 1], F16, name=f"v{tt}") for tt in range(TT)]
        for tt in range(TT):
            ps = psP.tile([128, 512], F32, tag="p")
            for d in range(DT):
                nc.tensor.matmul(
                    ps,
                    lhsT=xT(d, tt * 128, (tt + 1) * 128),
                    rhs=wvs[d],
                    start=(d == 0), stop=(d == DT - 1),
                )
            nc.gpsimd.memset(v_t[tt][:, :, DH : DH + 1], 1.0)
            nc.vector.tensor_add(
                out=v_t[tt][:, :, 0:DH],
                in0=ps.rearrange("p (h d) -> p h d", d=DH),
                in1=bv_sb,
            )
        filler = []
        for mt in range(1, MT):
            filler.extend(qk_proj_steps(mt))
        fstate = {"i": 0}

        def pull_filler(n):
            while fstate["i"] < len(filler) and n > 0:
                filler[fstate["i"]]()
                fstate["i"] += 1
                n -= 1

        def flush_filler_through(mt):
            """Emit all remaining filler steps for head-rows <= mt."""
            target = mt * len(filler) // (MT - 1)
            while fstate["i"] < target:
                filler[fstate["i"]]()
                fstate["i"] += 1

        # ---- phase 3: attention per (head, 512-wide i-block) ----
        yT_t = [ypool.tile([128, T], F16, name=f"yT{i}") for i in range(MT)]
        for h in range(NH):
            qrow = h // 2
            roff = (h % 2) * 64
            if qrow >= 1:
                flush_filler_through(qrow)
            kT_ap = qkT_t[MT + qrow]
            qT_ap = qkT_t[qrow]
            for ib in range(4):
                jlast = 4 * ib + 3
                Ups = psU.tile([128, 512], F32, tag="u")
                pend = []  # (jt, pt_tile, half_off, c0)
                for j0 in range(0, jlast + 1, 2):
                    ps = psS.tile([128, 1024], F32, tag="s")
                    pt = pts.tile([128, 1024], F16, tag="pt")
                    c0s = []
                    for half, jj in ((0, j0), (512, j0 + 1)):
                        k = jj - 4 * ib
                        c0 = 128 * k if k > 0 else 0
                        c0s.append(c0)
                        nc.tensor.matmul(
                            ps[:, half + c0 : half + 512],
                            lhsT=kT_ap[roff : roff + 64, jj * 128 : (jj + 1) * 128],
                            rhs=qT_ap[roff : roff + 64, ib * 512 + c0 : (ib + 1) * 512],
                            start=True, stop=True,
                        )
                    nc.scalar.activation(
                        out=pt[:, c0s[0] : 1024],
                        in_=ps[:, c0s[0] : 1024],
                        func=mybir.ActivationFunctionType.Exp,
                        bias=0.0, scale=1.0,
                    )
                    # post-exp band multiply (rel bias + causal zeros), f16 SBUF
                    for half, jj in ((0, j0), (512, j0 + 1)):
                        k = jj - 4 * ib
                        if -1 <= k <= 3:
                            lo = max(0, 128 * k)
                            hi = min(512, 128 * k + 256)
                            eb0 = h * 256 + (0 if k >= 0 else 128)
                            nc.vector.tensor_mul(
                                out=pt[:, half + lo : half + hi],
                                in0=pt[:, half + lo : half + hi],
                                in1=ebandT_sb[:, eb0 : eb0 + (hi - lo)],
                            )
                    pend.append((j0, pt, 0, c0s[0]))
                    pend.append((j0 + 1, pt, 512, c0s[1]))
                    pull_filler(2)
                    while len(pend) > 2:
                        jj, apt, half, cc = pend.pop(0)
                        nc.tensor.matmul(
                            Ups[0 : DH + 1, cc:512],
                            lhsT=v_t[jj][:, h, :],
                            rhs=apt[:, half + cc : half + 512],
                            start=(jj == 0), stop=False,
                            skip_group_check=True,
                        )
                for jj, apt, half, cc in pend:
                    nc.tensor.matmul(
                        Ups[0 : DH + 1, cc:512],
                        lhsT=v_t[jj][:, h, :],
                        rhs=apt[:, half + cc : half + 512],
                        start=(jj == 0), stop=(jj == jlast),
                        skip_group_check=True,
                    )
                rz = rzp.tile([1, 512], F32, tag="rz")
                nc.vector.reciprocal(out=rz, in_=Ups[DH : DH + 1, :])
                rzb = rzbp.tile([64, 512], F32, tag="rzb")
                nc.gpsimd.partition_broadcast(rzb, rz)
                nc.vector.tensor_mul(
                    out=yT_t[qrow][roff : roff + 64, ib * 512 : (ib + 1) * 512],
                    in0=Ups[0:DH, :],
                    in1=rzb,
                )
        flush_filler_through(MT - 1)

        # ---- phase 4: output projection (partial; host adds residual) ----
        for tt in range(TT):
            for oc in range(2):
                ps = psP.tile([128, 512], F32, tag="p")
                for kt in range(MT):
                    nc.tensor.matmul(
                        ps,
                        lhsT=yT_t[kt][:, tt * 128 : (tt + 1) * 128],
                        rhs=wos[kt][:, oc * 512 : (oc + 1) * 512],
                        start=(kt == 0), stop=(kt == MT - 1),
                    )
                osb = outp.tile([128, 512], F32, tag="o")
                if oc == 0:
                    nc.scalar.copy(out=osb, in_=ps)
                else:
                    nc.vector.tensor_copy(out=osb, in_=ps)
                nc.sync.dma_start(
                    out=out_d[tt * 128 : (tt + 1) * 128, oc * 512 : (oc + 1) * 512],
                    in_=osb,
                )

    nc.compile()
    return nc


def _host_prep(inputs):
    """Build the 8 per-core input maps."""
    x = np.asarray(inputs["x"], dtype=np.float32)
    Wq = np.asarray(inputs["Wq"], dtype=np.float32)
    Wk = np.asarray(inputs["Wk"], dtype=np.float32)
    Wv = np.asarray(inputs["Wv"], dtype=np.float32)
    Wo = np.asarray(inputs["Wo"], dtype=np.float32)
    rel = np.asarray(inputs["rel"], dtype=np.float32)
    gamma = np.asarray(inputs["ln_gamma"], dtype=np.float32)
    beta = np.asarray(inputs["ln_beta"], dtype=np.float32)

    sc = 1.0 / math.sqrt(DH)
    ii = np.arange(128)

    half = {}
    for hh in range(2):
        hs = slice(hh * M, (hh + 1) * M)
        Wq_h, Wk_h, Wv_h, Wo_h = Wq[hs], Wk[hs], Wv[hs], Wo[:, hs]
        wqT = np.ascontiguousarray((Wq_h * gamma[None, :] * sc).T).astype(np.float16)
        wkT = np.ascontiguousarray((Wk_h * gamma[None, :]).T).astype(np.float16)
        wvT = np.ascontiguousarray((Wv_h * gamma[None, :]).T).astype(np.float16)
        woT = np.ascontiguousarray(Wo_h.T).astype(np.float16)
        bq = ((Wq_h @ beta) * sc).reshape(MT, 128).T.astype(np.float32)
        bk = (Wk_h @ beta).reshape(MT, 128).T.astype(np.float32)
        bv = np.tile((Wv_h @ beta)[None, :], (128, 1)).reshape(128, NH, DH)

        # transposed multiplicative band tiles [j, i]:
        #   diag (it==jt):    exp(rel[i-j] - rel[128]) for i>=j, 0 above diag
        #   offdiag (it==jt+1): exp(rel[min(i-j+128,128)] - rel[128])
        ebandT = np.zeros((128, NH * 256), dtype=np.float32)
        d0 = ii[None, :] - ii[:, None]  # [j, i] = i - j
        for h in range(NH):
            g = hh * NH + h
            r128 = rel[g, 128]
            diag = np.where(d0 >= 0, np.exp(rel[g, np.clip(d0, 0, 128)] - r128), 0.0)
            offd = np.exp(rel[g, np.minimum(d0 + 128, 128)] - r128)
            ebandT[:, h * 256 : h * 256 + 128] = diag
            ebandT[:, h * 256 + 128 : h * 256 + 256] = offd
        half[hh] = dict(
            wqT=wqT, wkT=wkT, wvT=wvT, woT=woT,
            bq=np.ascontiguousarray(bq), bk=np.ascontiguousarray(bk),
            bv=bv.astype(np.float16),
            ebandT=ebandT.astype(np.float16),
        )

    in_maps = []
    for c in range(NCORES):
        b, hh = c // 2, c % 2
        m = dict(half[hh])
        m["x"] = np.ascontiguousarray(x[b])
        in_maps.append(m)
    return in_maps, x


def kernel(**inputs) -> np.ndarray:
    global _CACHED_NC
    if _CACHED_NC is None:
        _CACHED_NC = build_nc()
    nc = _CACHED_NC
    in_maps, x = _host_prep(inputs)
    res = run_bass_kernel_spmd(nc, in_maps, core_ids=list(range(NCORES)))
    out = np.empty_like(x)
    for b in range(4):
        out[b] = x[b] + res.results[2 * b]["out"] + res.results[2 * b + 1]["out"]
    return out


if __name__ == "__main__":
    rng = np.random.default_rng(0)
    fake = {
        "x": rng.standard_normal((4, T, D), dtype=np.float32),
        "Wq": rng.standard_normal((D, D), dtype=np.float32) / 32,
        "Wk": rng.standard_normal((D, D), dtype=np.float32) / 32,
        "Wv": rng.standard_normal((D, D), dtype=np.float32) / 32,
        "Wo": rng.standard_normal((D, D), dtype=np.float32) / 32,
        "rel": np.tile(np.linspace(0, -2, 129, dtype=np.float32), (16, 1)),
        "ln_gamma": np.ones(D, np.float32),
        "ln_beta": np.zeros(D, np.float32),
    }
    y = kernel(**fake)
    print("ran ok", y.shape, y.dtype)
